# revision 28
# baseline (speedup 1.0000x reference)
"""AdaptFormer (4-layer video ViT w/ parallel adapters) on 8 TRN2 NeuronCores.

Sharding: 2 batch elements x 4-way token sharding (392 tokens/core).
Per-layer AllGather of K^T and V (bf16) within each 4-core group.
All activations feature-major ([C_partition, token_free]); matmuls in bf16
with fp32 PSUM accumulate; softmax exp on ScalarE; LN via ones-matmul
column sums + outer-product broadcast matmuls.

Self-contained: hardcodes all shapes; host-side prep does sharding,
im2col for the patch conv, weight transpose/pre-tiling, and bf16 casts.
"""
import math
import os
import numpy as np
import ml_dtypes

import concourse.bass as bass
import concourse.tile as tile
from concourse import bacc, mybir
from concourse.bass_utils import run_bass_kernel_spmd

BF16 = mybir.dt.bfloat16
F32 = mybir.dt.float32
AF = mybir.ActivationFunctionType

NCORES = 8
GROUPS = [[0, 1, 2, 3], [4, 5, 6, 7]]
DEPTH = 4
C = 768
NH = 12
HD = 64
FF = 3072
BOT = 64
ADA = 0.1
EPS = 1e-5
N = 1568            # tokens per batch element (8*14*14)
S = 392             # tokens per core
NJ = C // 128       # 6 feature tiles
NF = FF // 128      # 24
KIN = 1536          # conv patch dim (3*2*16*16)
NKC = KIN // 128    # 12
NKT = (N + 127) // 128   # 13 global k-token tiles (last has 32 rows)
LAST_KT = N - 12 * 128   # 32

_NC_CACHE = {}
LAST_RESULT = None


def _mm(nc, out, lhsT, rhs, start, stop, tp=None):
    if tp is None:
        nc.tensor.matmul(out, lhsT, rhs, start=start, stop=stop)
    else:
        nc.tensor.matmul(out, lhsT, rhs, start=start, stop=stop, tile_position=tp)


def build_nc():
    stage = int(os.environ.get("KSTAGE", "45"))
    nc = bacc.Bacc(None, num_devices=NCORES)

    # ---- DRAM parameters (per-core shards / replicated weights) ----
    P = {}
    P["xim"] = nc.declare_dram_parameter("xim", [128, NKC, S], BF16, isOutput=False)
    P["convw"] = nc.declare_dram_parameter("convw", [128, NKC, C], BF16, isOutput=False)
    P["pos"] = nc.declare_dram_parameter("pos", [128, NJ, S], F32, isOutput=False)
    # col biases: per layer [128, 48]: qb 0-5, projb 6-11, fc2b 12-17, upb01 18-23, fc1b 24-47
    P["cbias"] = nc.declare_dram_parameter("cbias", [128, DEPTH, 48], F32, isOutput=False)
    P["dbias"] = nc.declare_dram_parameter("dbias", [64, DEPTH], F32, isOutput=False)
    # column-layout LN params f32: per layer rows (ln1_g, ln1_b, ada_g, ada_b, ln2_g, ln2_b)
    P["lncol"] = nc.declare_dram_parameter("lncol", [128, DEPTH, 6, NJ], F32, isOutput=False)
    P["fncol"] = nc.declare_dram_parameter("fncol", [128, 2, NJ], F32, isOutput=False)
    for l in range(DEPTH):
        P[f"wq{l}"] = nc.declare_dram_parameter(f"wq{l}", [128, NJ, C], BF16, isOutput=False)
        P[f"wk{l}"] = nc.declare_dram_parameter(f"wk{l}", [128, NJ, C], BF16, isOutput=False)
        P[f"wv{l}"] = nc.declare_dram_parameter(f"wv{l}", [128, NJ, C], BF16, isOutput=False)
        P[f"wp{l}"] = nc.declare_dram_parameter(f"wp{l}", [128, NJ, C], BF16, isOutput=False)
        P[f"f1a{l}"] = nc.declare_dram_parameter(f"f1a{l}", [128, NJ, FF // 2], BF16, isOutput=False)
        P[f"f1b{l}"] = nc.declare_dram_parameter(f"f1b{l}", [128, NJ, FF // 2], BF16, isOutput=False)
        P[f"f2a{l}"] = nc.declare_dram_parameter(f"f2a{l}", [128, NF, C // 2], BF16, isOutput=False)
        P[f"f2b{l}"] = nc.declare_dram_parameter(f"f2b{l}", [128, NF, C // 2], BF16, isOutput=False)
        P[f"dn{l}"] = nc.declare_dram_parameter(f"dn{l}", [128, NJ, BOT], BF16, isOutput=False)
        P[f"up{l}"] = nc.declare_dram_parameter(f"up{l}", [64, C], BF16, isOutput=False)
    out_ext = nc.declare_dram_parameter("out", [128, NJ, S], F32, isOutput=True)

    with tile.TileContext(nc) as tc:
        import contextlib
        ctx = contextlib.ExitStack()
        with ctx:
            # ---------------- pools ----------------
            const = ctx.enter_context(tc.tile_pool(name="const", bufs=1))
            persist = ctx.enter_context(tc.tile_pool(name="persist", bufs=1))
            hpool = ctx.enter_context(tc.tile_pool(name="hpool", bufs=3))
            big_pool = ctx.enter_context(tc.tile_pool(name="big", bufs=2))
            wp_pool = ctx.enter_context(tc.tile_pool(name="wp", bufs=3))
            wmlp_pool = ctx.enter_context(tc.tile_pool(name="wmlp", bufs=2))
            wada_pool = ctx.enter_context(tc.tile_pool(name="wada", bufs=2))
            cast_pool = ctx.enter_context(tc.tile_pool(name="cast", bufs=6))
            tmp_pool = ctx.enter_context(tc.tile_pool(name="tmpf", bufs=4))
            small_pool = ctx.enter_context(tc.tile_pool(name="small", bufs=8))
            ppool = ctx.enter_context(tc.tile_pool(name="ppool", bufs=8))
            ps_mm = ctx.enter_context(tc.tile_pool(name="ps_mm", bufs=2, space="PSUM"))
            ps_s = ctx.enter_context(tc.tile_pool(name="ps_s", bufs=2, space="PSUM"))
            ps_o = ctx.enter_context(tc.tile_pool(name="ps_o", bufs=2, space="PSUM"))
            dram = ctx.enter_context(tc.tile_pool(name="dram", bufs=1, space="DRAM"))

            # ---------------- constants ----------------
            ones_bf = const.tile([128, S], BF16)
            nc.vector.memset(ones_bf, 1.0)
            ceps_sb = const.tile([128, 1], F32)
            nc.vector.memset(ceps_sb, C * EPS)
            hlc_sb = const.tile([128, 1], F32)
            nc.vector.memset(hlc_sb, 0.5 * math.log(C))
            cbias = const.tile([128, DEPTH, 48], F32)
            nc.sync.dma_start(out=cbias, in_=P["cbias"][:, :, :])
            dbias = const.tile([64, DEPTH], F32)
            nc.sync.dma_start(out=dbias, in_=P["dbias"][:, :])
            lncol = const.tile([128, DEPTH, 6, NJ], F32)
            nc.sync.dma_start(out=lncol, in_=P["lncol"][:, :, :, :])
            fncol = const.tile([128, 2, NJ], F32)
            nc.sync.dma_start(out=fncol, in_=P["fncol"][:, :, :])

            # residual tensors (alternate per layer) + persistent activations
            tokA = persist.tile([128, NJ, S], F32, name="tokA")
            tokB = persist.tile([128, NJ, S], F32, name="tokB")
            q_sb = persist.tile([128, NJ, S], BF16, name="q_sb")
            k_loc = persist.tile([128, NJ, S], BF16, name="k_loc")
            v_loc = persist.tile([128, 4, NH, HD + 1], BF16, name="v_loc")
            o_sb = persist.tile([128, NJ, S], BF16, name="o_sb")

            nc.vector.memset(v_loc[:, :, :, HD:HD + 1], 1.0)

            # DRAM bounce buffers for the per-layer AllGather (k^T then v)
            VW = NH * (HD + 1)              # 780: v row width incl ones col
            KSZ = NJ * 128 * S
            VSZ = S * VW
            AGE = KSZ + VSZ
            ag_in = dram.tile([AGE], BF16, name="ag_in")
            ag_out = dram.tile([4, AGE], BF16, name="ag_out")
            ag_wu_i = dram.tile([256], BF16, name="ag_wu_i")
            ag_wu_o = dram.tile([4, 256], BF16, name="ag_wu_o")

            # ---------------- LN helper ----------------
            def layer_norm(tok, gcol, bcol, out_t):
                """Feature-major LN. tok f32 [128, NJ, S]; gcol/bcol(j) -> [128,1] f32 APs.
                out_t[:, j, :] = ((tok - m) * rstd) * g + b, in out_t's dtype."""
                stat = ps_s.tile([128, S], F32, tag="ps_s")
                tbs = []
                for j in range(NJ):
                    tb = cast_pool.tile([128, S], BF16, tag="castln", bufs=6)
                    nc.vector.tensor_copy(out=tb, in_=tok[:, j, :])
                    tbs.append(tb)
                    sq = cast_pool.tile([128, S], BF16, tag="cast")
                    nc.scalar.activation(out=sq, in_=tok[:, j, :], func=AF.Square)
                    _mm(nc, stat[0:1, :], ones_bf[:, 0:1], tb, j == 0, j == NJ - 1, (0, 0))
                    _mm(nc, stat[64:65, :], ones_bf[:, 0:1], sq, j == 0, j == NJ - 1, (0, 64))
                # m = sum/C; vu = sumsq - m*sum (= C*var); rstd = exp(-(ln(vu+C*eps)-lnC)/2)
                m_sb = small_pool.tile([1, S], F32, tag="small")
                nc.vector.tensor_scalar(out=m_sb, in0=stat[0:1, :], scalar1=1.0 / C,
                                        scalar2=None, op0=mybir.AluOpType.mult)
                t_sb = small_pool.tile([1, S], F32, tag="small")
                nc.vector.tensor_mul(out=t_sb, in0=m_sb, in1=stat[0:1, :])
                vu = small_pool.tile([1, S], F32, tag="small")
                nc.vector.tensor_sub(out=vu, in0=stat[64:65, :], in1=t_sb)
                lnv = small_pool.tile([1, S], F32, tag="small")
                nc.scalar.activation(out=lnv, in_=vu, func=AF.Ln, bias=ceps_sb[0:1, 0:1],
                                     scale=1.0)
                rstd_bf = small_pool.tile([1, S], BF16, tag="smallb")
                nc.scalar.activation(out=rstd_bf, in_=lnv, func=AF.Exp, bias=hlc_sb[0:1, 0:1],
                                     scale=-0.5)
                u_bf = small_pool.tile([1, S], BF16, tag="smallb")
                nc.vector.tensor_mul(out=u_bf, in0=m_sb, in1=rstd_bf)
                r_ps = ps_mm.tile([128, S], F32, tag="ps_mm")
                _mm(nc, r_ps, ones_bf[0:1, 0:128], rstd_bf, True, True)
                u_ps = ps_mm.tile([128, S], F32, tag="ps_mm")
                _mm(nc, u_ps, ones_bf[0:1, 0:128], u_bf, True, True)
                r_sb = cast_pool.tile([128, S], BF16, tag="cast")
                nc.vector.tensor_copy(out=r_sb, in_=r_ps)
                u_sb = cast_pool.tile([128, S], BF16, tag="cast")
                nc.vector.tensor_copy(out=u_sb, in_=u_ps)
                for j in range(NJ):
                    t1 = cast_pool.tile([128, S], BF16, tag="cast")
                    nc.vector.tensor_mul(out=t1, in0=tbs[j], in1=r_sb)
                    t2 = cast_pool.tile([128, S], BF16, tag="cast")
                    nc.vector.tensor_sub(out=t2, in0=t1, in1=u_sb)
                    nc.scalar.activation(out=out_t[:, j, :], in_=t2, func=AF.Identity,
                                         bias=bcol(j), scale=gcol(j))

            # warm up the collective path while conv runs
            wu_sb = const.tile([1, 256], BF16)
            nc.vector.memset(wu_sb, 0.0)
            nc.sync.dma_start(out=ag_wu_i[:].rearrange("(p c) -> p c", p=1), in_=wu_sb)
            nc.gpsimd.collective_compute(
                "AllGather", mybir.AluOpType.bypass, replica_groups=GROUPS,
                ins=[ag_wu_i.opt()], outs=[ag_wu_o.opt()])

            # ---------------- patch-embed conv ----------------
            xim_sb = wp_pool.tile([128, NKC, S], BF16, tag="wp")
            nc.sync.dma_start(out=xim_sb, in_=P["xim"][:, :, :])
            convw_sb = wmlp_pool.tile([128, NKC, C], BF16, tag="wmlp")
            nc.sync.dma_start(out=convw_sb, in_=P["convw"][:, :, :])
            pos_sb = wp_pool.tile([128, NJ, S], F32, tag="wp")
            nc.sync.dma_start(out=pos_sb, in_=P["pos"][:, :, :])
            for mo in range(NJ):
                ps = ps_mm.tile([128, S], F32, tag="ps_mm")
                for ki in range(NKC):
                    _mm(nc, ps, convw_sb[:, ki, mo * 128:(mo + 1) * 128],
                        xim_sb[:, ki, :], ki == 0, ki == NKC - 1)
                nc.vector.tensor_add(out=tokA[:, mo, :], in0=ps, in1=pos_sb[:, mo, :])

            # ---------------- transformer layers ----------------
            # KSTAGE = 10*full_layers + substage (dev bisect); default 100 = everything
            nfull = min(DEPTH, stage // 10)
            sub = stage % 10
            for l in range(min(DEPTH, nfull + 1)):
                partial = l >= nfull
                cur = tokA if l % 2 == 0 else tokB
                nxt = tokB if l % 2 == 0 else tokA
                qb = cbias[:, l, 0:1]
                projb = lambda j: cbias[:, l, 6 + j:7 + j]
                fc2b = lambda j: cbias[:, l, 12 + j:13 + j]
                upb = lambda j: cbias[:, l, 18 + j:19 + j]
                fc1b = lambda j: cbias[:, l, 24 + j:25 + j]
                qb_j = lambda j: cbias[:, l, 0 + j:1 + j]
                lnp = lambda r: (lambda j: lncol[:, l, r, j:j + 1])

                wq = wp_pool.tile([128, NJ, C], BF16, tag="wp")
                nc.sync.dma_start(out=wq, in_=P[f"wq{l}"][:, :, :])
                wk = wp_pool.tile([128, NJ, C], BF16, tag="wp")
                nc.sync.dma_start(out=wk, in_=P[f"wk{l}"][:, :, :])
                wv = wp_pool.tile([128, NJ, C], BF16, tag="wp")
                nc.sync.dma_start(out=wv, in_=P[f"wv{l}"][:, :, :])

                # LN1 -> h1
                h1 = hpool.tile([128, NJ, S], BF16, tag="h")
                layer_norm(cur, lnp(0), lnp(1), h1)

                # k^T (no bias)
                for mo in range(NJ):
                    ps = ps_mm.tile([128, S], F32, tag="ps_mm")
                    for ki in range(NJ):
                        _mm(nc, ps, wk[:, ki, mo * 128:(mo + 1) * 128], h1[:, ki, :],
                            ki == 0, ki == NJ - 1)
                    nc.vector.tensor_copy(out=k_loc[:, mo, :], in_=ps)
                nc.sync.dma_start(out=ag_in[0:KSZ].rearrange("(p j s) -> p j s", p=128, j=NJ),
                                  in_=k_loc)
                # v token-major: chunks of local tokens
                for tt in range(4):
                    rows = 128 if tt < 3 else S - 3 * 128  # 8 on the last chunk
                    tsl = slice(tt * 128, tt * 128 + rows)
                    for half in range(2):
                        ps = ps_mm.tile([128, 384], F32, tag="ps_mm")
                        for ki in range(NJ):
                            _mm(nc, ps[:rows, :], h1[:, ki, tsl],
                                wv[:, ki, half * 384:(half + 1) * 384], ki == 0, ki == NJ - 1)
                        nc.vector.tensor_copy(
                            out=v_loc[:rows, tt, half * 6:(half + 1) * 6, 0:HD],
                            in_=ps[:rows, :].rearrange("p (h d) -> p h d", h=6))
                nc.sync.dma_start(
                    out=ag_in[KSZ:KSZ + 384 * VW].rearrange("(t p c) -> p t c", p=128, t=3),
                    in_=v_loc[:, 0:3, :, :].rearrange("p t h d -> p t (h d)"))
                nc.sync.dma_start(
                    out=ag_in[KSZ + 384 * VW:KSZ + S * VW].rearrange("(p c) -> p c", p=8),
                    in_=v_loc[0:8, 3, :, :].rearrange("p h d -> p (h d)"))
                nc.gpsimd.collective_compute(
                    "AllGather", mybir.AluOpType.bypass, replica_groups=GROUPS,
                    ins=[ag_in.opt()], outs=[ag_out.opt()])

                # q^T (bias, scale already folded into weights host-side)
                for mo in range(NJ):
                    ps = ps_mm.tile([128, S], F32, tag="ps_mm")
                    for ki in range(NJ):
                        _mm(nc, ps, wq[:, ki, mo * 128:(mo + 1) * 128], h1[:, ki, :],
                            ki == 0, ki == NJ - 1)
                    nc.vector.tensor_scalar(out=q_sb[:, mo, :], in0=ps, scalar1=qb_j(mo),
                                            scalar2=None, op0=mybir.AluOpType.add)

                # unpack gathered k^T / v
                kT_full = big_pool.tile([128, NJ, N], BF16, tag="big")
                v_full = big_pool.tile([128, NKT, VW], BF16, tag="big")
                for rr in range(4):
                    nc.sync.dma_start(
                        out=kT_full[:, :, rr * S:(rr + 1) * S],
                        in_=ag_out[rr, 0:KSZ].rearrange("(p j s) -> p j s", p=128, j=NJ))
                vview = lambda rr: ag_out[rr, KSZ:KSZ + S * VW].rearrange("(t c) -> t c", c=VW)
                for rr in range(4):
                    g0 = rr * S
                    seg = []
                    g = g0
                    while g < g0 + S:
                        kt, p = divmod(g, 128)
                        take = min(128 - p, g0 + S - g)
                        seg.append((kt, p, g - g0, take))
                        g += take
                    # merge consecutive full tiles
                    i = 0
                    while i < len(seg):
                        kt, p, off, take = seg[i]
                        if p == 0 and take == 128:
                            n_full = 1
                            while (i + n_full < len(seg) and seg[i + n_full][1] == 0
                                   and seg[i + n_full][3] == 128):
                                n_full += 1
                            nc.sync.dma_start(
                                out=v_full[:, kt:kt + n_full, :],
                                in_=vview(rr)[off:off + n_full * 128, :].rearrange(
                                    "(t p) c -> p t c", p=128))
                            i += n_full
                        else:
                            nc.sync.dma_start(out=v_full[p:p + take, kt, :],
                                              in_=vview(rr)[off:off + take, :])
                            i += 1

                if partial and sub < 3:
                    break
                # ---- attention: 6 head pairs ----
                wproj = wp_pool.tile([128, NJ, C], BF16, tag="wp")
                nc.sync.dma_start(out=wproj, in_=P[f"wp{l}"][:, :, :])
                for hp in range(NJ):
                    o_ps0 = ps_o.tile([128, S], F32, tag="ps_o")
                    o_ps1 = ps_o.tile([128, S], F32, tag="ps_o")
                    for kt in range(NKT):
                        used = 128 if kt < NKT - 1 else LAST_KT
                        ksl = slice(kt * 128, kt * 128 + used)
                        s01 = ps_s.tile([128, 2, 512], F32, tag="ps_s")
                        _mm(nc, s01[:used, 0, 0:S], kT_full[0:64, hp, ksl], q_sb[0:64, hp, :],
                            True, True, (0, 0))
                        _mm(nc, s01[:used, 1, 0:S], kT_full[64:128, hp, ksl], q_sb[64:128, hp, :],
                            True, True, (64, 0))
                        p01 = ppool.tile([128, 2, S], BF16, tag="pp")
                        nc.scalar.activation(out=p01[:used, :, :], in_=s01[:used, :, 0:S],
                                             func=AF.Exp)
                        _mm(nc, o_ps0[0:65, :], v_full[:used, kt, hp * 130:hp * 130 + 65],
                            p01[:used, 0, :], kt == 0, kt == NKT - 1, (0, 0))
                        _mm(nc, o_ps1[0:65, :], v_full[:used, kt, hp * 130 + 65:hp * 130 + 130],
                            p01[:used, 1, :], kt == 0, kt == NKT - 1, (0, 0))
                    d0 = small_pool.tile([1, S], F32, tag="small")
                    nc.vector.tensor_copy(out=d0, in_=o_ps0[64:65, :])
                    d1 = small_pool.tile([1, S], F32, tag="small")
                    nc.vector.tensor_copy(out=d1, in_=o_ps1[64:65, :])
                    rd0 = small_pool.tile([1, S], F32, tag="small")
                    nc.vector.reciprocal_approx_fast(out=rd0, in_=d0)
                    rd1 = small_pool.tile([1, S], F32, tag="small")
                    nc.vector.reciprocal_approx_fast(out=rd1, in_=d1)
                    rd0b = small_pool.tile([1, S], BF16, tag="smallb")
                    nc.vector.tensor_copy(out=rd0b, in_=rd0)
                    rd1b = small_pool.tile([1, S], BF16, tag="smallb")
                    nc.vector.tensor_copy(out=rd1b, in_=rd1)
                    bc_ps = ps_mm.tile([128, S], F32, tag="ps_mm")
                    _mm(nc, bc_ps[0:64, :], ones_bf[0:1, 0:64], rd0b, True, True, (0, 0))
                    bc2_ps = ps_mm.tile([128, S], F32, tag="ps_mm")
                    _mm(nc, bc2_ps[0:64, :], ones_bf[0:1, 0:64], rd1b, True, True, (0, 0))
                    bc_sb = cast_pool.tile([64, S], BF16, tag="cast")
                    nc.vector.tensor_copy(out=bc_sb, in_=bc_ps[0:64, :])
                    bc2_sb = cast_pool.tile([64, S], BF16, tag="cast")
                    nc.vector.tensor_copy(out=bc2_sb, in_=bc2_ps[0:64, :])
                    nc.vector.tensor_mul(out=o_sb[0:64, hp, :], in0=o_ps0[0:64, :],
                                         in1=bc_sb[0:64, :])
                    otmp = cast_pool.tile([64, S], BF16, tag="cast")
                    nc.vector.tensor_mul(out=otmp[0:64, :], in0=o_ps1[0:64, :],
                                         in1=bc2_sb[0:64, :])
                    nc.sync.dma_start(out=o_sb[64:128, hp, :], in_=otmp[0:64, :])

                # proj + residual -> nxt
                for mo in range(NJ):
                    ps = ps_mm.tile([128, S], F32, tag="ps_mm")
                    for ki in range(NJ):
                        _mm(nc, ps, wproj[:, ki, mo * 128:(mo + 1) * 128], o_sb[:, ki, :],
                            ki == 0, ki == NJ - 1)
                    nc.vector.affine_then_add(out=nxt[:, mo, :], in0=ps, in1=cur[:, mo, :],
                                              scale=1.0, bias=projb(mo))

                if partial and sub < 4:
                    break
                # ---- parallel adapter ----
                dn = wada_pool.tile([128, NJ, BOT], BF16, tag="wada_dn")
                nc.sync.dma_start(out=dn, in_=P[f"dn{l}"][:, :, :])
                up = wada_pool.tile([64, C], BF16, tag="wada_up")
                nc.sync.dma_start(out=up, in_=P[f"up{l}"][:, :])
                ha = hpool.tile([128, NJ, S], BF16, tag="h")
                layer_norm(nxt, lnp(2), lnp(3), ha)
                adapt_sb = hpool.tile([128, NJ, S], BF16, tag="h")
                dps = ps_mm.tile([64, S], F32, tag="ps_mm")
                for ki in range(NJ):
                    _mm(nc, dps, dn[:, ki, :], ha[:, ki, :], ki == 0, ki == NJ - 1)
                a1 = cast_pool.tile([64, S], BF16, tag="cast")
                nc.scalar.activation(out=a1, in_=dps, func=AF.Relu, bias=dbias[:, l:l + 1],
                                     scale=1.0)
                for mo in range(NJ):
                    ups = ps_mm.tile([128, S], F32, tag="ps_mm")
                    _mm(nc, ups, up[:, mo * 128:(mo + 1) * 128], a1, True, True)
                    nc.scalar.activation(out=adapt_sb[:, mo, :], in_=ups, func=AF.Identity,
                                         bias=upb(mo), scale=ADA)

                if partial and sub < 5:
                    break
                # ---- MLP ----
                h2 = hpool.tile([128, NJ, S], BF16, tag="h")
                layer_norm(nxt, lnp(4), lnp(5), h2)
                m1_sb = big_pool.tile([128, NF, S], BF16, tag="big")
                for half in range(2):
                    f1h = wmlp_pool.tile([128, NJ, FF // 2], BF16, tag="wmlp")
                    nc.sync.dma_start(out=f1h, in_=P[f"f1{'ab'[half]}{l}"][:, :, :])
                    for mo12 in range(12):
                        mo = half * 12 + mo12
                        ps = ps_mm.tile([128, S], F32, tag="ps_mm")
                        for ki in range(NJ):
                            _mm(nc, ps, f1h[:, ki, mo12 * 128:(mo12 + 1) * 128], h2[:, ki, :],
                                ki == 0, ki == NJ - 1)
                        nc.scalar.activation(out=m1_sb[:, mo, :], in_=ps, func=AF.Gelu,
                                             bias=fc1b(mo), scale=1.0)
                # nxt + adapt -> cur (cur is dead; becomes the mlp residual base)
                for mo in range(NJ):
                    nc.vector.tensor_add(out=cur[:, mo, :], in0=nxt[:, mo, :],
                                         in1=adapt_sb[:, mo, :])
                for half in range(2):
                    f2h = wmlp_pool.tile([128, NF, C // 2], BF16, tag="wmlp")
                    nc.sync.dma_start(out=f2h, in_=P[f"f2{'ab'[half]}{l}"][:, :, :])
                    for mo3 in range(3):
                        mo = half * 3 + mo3
                        ps = ps_mm.tile([128, S], F32, tag="ps_mm")
                        for ki in range(NF):
                            _mm(nc, ps, f2h[:, ki, mo3 * 128:(mo3 + 1) * 128], m1_sb[:, ki, :],
                                ki == 0, ki == NF - 1)
                        nc.vector.affine_then_add(out=nxt[:, mo, :], in0=ps, in1=cur[:, mo, :],
                                                  scale=1.0, bias=fc2b(mo))

            # ---------------- final LN -> output ----------------
            if stage >= 40 + 5:
                fin = tokA if DEPTH % 2 == 0 else tokB
                out_sb = tokB if DEPTH % 2 == 0 else tokA
                layer_norm(fin, lambda j: fncol[:, 0, j:j + 1], lambda j: fncol[:, 1, j:j + 1], out_sb)
                nc.sync.dma_start(out=out_ext[:, :, :], in_=out_sb)
            else:
                nc.sync.dma_start(out=out_ext[:, :, :], in_=tokA)

    nc.compile()
    return nc


# ======================= host-side prep =======================

def _tile_rows(a):
    """[K, M] -> [128, K//128, M] (partition-tiled, contiguous for DMA)."""
    K, M = a.shape
    assert K % 128 == 0
    return np.ascontiguousarray(a.reshape(K // 128, 128, M).transpose(1, 0, 2))


def _sinusoid(n, d):
    pos = np.arange(n)[:, None].astype(np.float64)
    i = np.arange(d)[None, :]
    angle = pos / np.power(10000.0, 2.0 * (i // 2) / d)
    tab = np.zeros((n, d))
    tab[:, 0::2] = np.sin(angle[:, 0::2])
    tab[:, 1::2] = np.cos(angle[:, 1::2])
    return tab.astype(np.float32)  # [N, C]


def prep_inputs(inputs):
    f32 = np.float32
    bf = ml_dtypes.bfloat16
    x = np.asarray(inputs["x"], f32)
    conv_w = np.asarray(inputs["conv_w"], f32)
    conv_b = np.asarray(inputs["conv_b"], f32)
    scale = HD ** -0.5

    pos_full = _sinusoid(N, C) + conv_b[None, :]          # [N, C] with conv bias folded
    convw_t = conv_w.reshape(C, KIN).T                    # [1536, 768]
    convw_tiled = _tile_rows(convw_t).astype(bf)

    shared = {"convw": convw_tiled}
    shared["cbias"] = np.zeros((128, DEPTH, 48), f32)
    shared["dbias"] = np.zeros((64, DEPTH), f32)
    shared["lncol"] = np.zeros((128, DEPTH, 6, NJ), f32)
    shared["fncol"] = np.zeros((128, 2, NJ), f32)
    for l in range(DEPTH):
        qb = np.asarray(inputs["q_b"], f32)[l] * scale
        vb = np.asarray(inputs["v_b"], f32)[l]
        pw = np.asarray(inputs["proj_w"], f32)[l]
        pb = np.asarray(inputs["proj_b"], f32)[l] + pw @ vb   # fold v bias into proj bias
        f2b = np.asarray(inputs["fc2_b"], f32)[l]
        upb = np.asarray(inputs["up_b"], f32)[l] * ADA
        f1b = np.asarray(inputs["fc1_b"], f32)[l]
        cb = shared["cbias"]
        cb[:, l, 0:6] = qb.reshape(6, 128).T
        cb[:, l, 6:12] = pb.reshape(6, 128).T
        cb[:, l, 12:18] = f2b.reshape(6, 128).T
        cb[:, l, 18:24] = upb.reshape(6, 128).T
        cb[:, l, 24:48] = f1b.reshape(24, 128).T
        shared["dbias"][:, l] = np.asarray(inputs["down_b"], f32)[l]
        lr = shared["lncol"]
        for r, key in enumerate(["ln1_g", "ln1_b", "ada_g", "ada_b", "ln2_g", "ln2_b"]):
            lr[:, l, r, :] = np.asarray(inputs[key], f32)[l].reshape(NJ, 128).T
        shared[f"wq{l}"] = _tile_rows(np.asarray(inputs["q_w"], f32)[l].T * scale).astype(bf)
        shared[f"wk{l}"] = _tile_rows(np.asarray(inputs["k_w"], f32)[l].T).astype(bf)
        shared[f"wv{l}"] = _tile_rows(np.asarray(inputs["v_w"], f32)[l].T).astype(bf)
        shared[f"wp{l}"] = _tile_rows(pw.T).astype(bf)
        f1t = _tile_rows(np.asarray(inputs["fc1_w"], f32)[l].T).astype(bf)   # [128, NJ, FF]
        shared[f"f1a{l}"] = np.ascontiguousarray(f1t[:, :, :FF // 2])
        shared[f"f1b{l}"] = np.ascontiguousarray(f1t[:, :, FF // 2:])
        f2t = _tile_rows(np.asarray(inputs["fc2_w"], f32)[l].T).astype(bf)   # [128, NF, C]
        shared[f"f2a{l}"] = np.ascontiguousarray(f2t[:, :, :C // 2])
        shared[f"f2b{l}"] = np.ascontiguousarray(f2t[:, :, C // 2:])
        shared[f"dn{l}"] = _tile_rows(np.asarray(inputs["down_w"], f32)[l].T).astype(bf)
        shared[f"up{l}"] = np.asarray(inputs["up_w"], f32)[l].T.astype(bf)  # [64, 768]
    shared["fncol"][:, 0, :] = np.asarray(inputs["normf_g"], f32).reshape(NJ, 128).T
    shared["fncol"][:, 1, :] = np.asarray(inputs["normf_b"], f32).reshape(NJ, 128).T

    in_maps = []
    for core in range(NCORES):
        b, r = divmod(core, 4)
        m = dict(shared)
        # im2col: tokens (d,h,w) for d in [2r, 2r+2); patch dims (c, dd, hh, ww)
        xs = x[b, :, 4 * r:4 * r + 4, :, :]               # [3, 4, 224, 224]
        xs = xs.reshape(3, 2, 2, 14, 16, 14, 16)          # c, d, dd, h, hh, w, ww
        xim = xs.transpose(0, 2, 4, 6, 1, 3, 5).reshape(KIN, S)
        m["xim"] = _tile_rows(xim).astype(bf)
        m["pos"] = _tile_rows(
            np.ascontiguousarray(pos_full[r * S:(r + 1) * S, :].T))  # [128, 6, 392]
        in_maps.append(m)
    return in_maps


def kernel(**inputs):
    global LAST_RESULT
    if "nc" not in _NC_CACHE:
        _NC_CACHE["nc"] = build_nc()
    nc = _NC_CACHE["nc"]
    in_maps = prep_inputs(inputs)
    res = run_bass_kernel_spmd(nc, in_maps, core_ids=list(range(NCORES)), trace=False)
    LAST_RESULT = res
    out = np.empty((2, N, C), np.float32)
    for core in range(NCORES):
        b, r = divmod(core, 4)
        o = res.results[core]["out"]                      # [128, 6, 392]
        out[b, r * S:(r + 1) * S, :] = o.transpose(2, 1, 0).reshape(S, C)
    return out
